# revision 9
# baseline (speedup 1.0000x reference)
"""Trainium2 Bass kernel for the BMoIE (dense mixture-of-experts) network.

Network (per sample):
    alpha = softmax(x @ gate_w + gate_b)                       # [B, 8]
    h = relu(sum_k alpha_k * (h @ w_l[k] + b_l[k]))            # 3 hidden blocks
    out = sum_k alpha_k * (h @ wo[k] + bo[k])                  # output block

Strategy: data-parallel over 8 NeuronCores (2048 rows each, SPMD, no
collectives). Per core, activations are kept FEATURE-MAJOR the whole way
(hT[ci] = [128 feat x 2048 batch]) and the per-sample alpha is folded into
the activations once per layer:

    AT[(k,ci)] = alpha_bc[k] * hT[ci]          (VectorE, bf16)
    outT[co]  += W[k,ci,co]^T @ AT[(k,ci)]     (PE, accumulate in PSUM)

which turns each MoIE block into ONE dense GEMM with contraction K*D=4096
(32 accumulation steps per PSUM bank) — no per-expert combine pass and no
transposes between layers. ReLU is fused into the PSUM->SBUF eviction
(ScalarE), which writes the next layer's bf16 hT directly.

The layer is processed in 4 column-passes of 512 batch columns; each pass
accumulates the 4 output chunks in 4 PSUM banks while the previous pass's
4 banks drain through ScalarE, so the PE stream never waits on evictions
(avoids the HAM clock-gate re-throttle that cost the batch-major version
~120us). All matmul operands are bf16 (full PE speed; LDWEIGHTS for a
bf16 stationary is 64 cyc and hidden behind the 512-cycle matmuls).

alpha is computed batch-major (baseline gating code), transposed to
alphaT[8, 2048] with PE transposes, then broadcast across partitions with
ones[1,128]^T @ alphaT[k] PE matmuls into alpha_bc[k] = [128, 2048] bf16.
Per-layer biases (zero in the graded problem, supported anyway) enter as
the first accumulation step: acc[co] = bl[li][:, co]^T @ alphaT_bf.
"""

import sys

sys.path.insert(0, "/opt/trn_rl_repo")

import numpy as np

import concourse.bass as bass
import concourse.mybir as mybir
import concourse.tile as tile
from concourse import bacc
from concourse.bass_utils import run_bass_kernel_spmd
from concourse.masks import make_identity

P = 128           # partitions
D = 512           # model dim (= hidden dim)
K = 8             # experts
NCORES = 8
B = 16384
R = B // NCORES   # rows per core (batch columns in feature-major layout)
NT = R // P       # 16 batch tiles per core
NCH = D // P      # 4 feature chunks (both input ci and output co)
CB = 512          # batch columns per pass (PSUM bank = 512 fp32)
NPASS = R // CB   # 4 passes per layer
F32 = mybir.dt.float32
BF = mybir.dt.bfloat16
AF = mybir.ActivationFunctionType
ALU = mybir.AluOpType

W_NAMES = ("w0", "w1", "w2", "wo")
B_NAMES = ("b0", "b1", "b2", "bo")


def _build(has_gate_b, has_bias, repeat=1, mode="full", at_bufs=8, w_bufs=48,
           gp_at=0):
    """Trace + compile the per-core kernel. has_bias is a 4-tuple of bools.

    repeat>1 runs the whole 4-layer stack that many times (same weights,
    full DMA traffic each time) — used only for timing measurements.
    mode: "full" | "pe_same" (AT generated once per pass and reused by all
    32 accumulation steps — isolates PE throughput; wrong results).
    gp_at: number of experts (out of 8) whose AT scaling runs on GpSimd
    instead of VectorE.
    """
    from contextlib import ExitStack

    nc = bacc.Bacc("TRN2", target_bir_lowering=False, num_devices=NCORES)
    x = nc.dram_tensor("x", [R, D], F32, kind="ExternalInput")
    gate_w = nc.dram_tensor("gate_w", [D, K], BF, kind="ExternalInput")
    gate_b = nc.dram_tensor("gate_b", [K], F32, kind="ExternalInput")
    ws = [nc.dram_tensor(n, [K, D, D], BF, kind="ExternalInput") for n in W_NAMES]
    bs = [nc.dram_tensor(n, [K, D], BF, kind="ExternalInput") for n in B_NAMES]
    out = nc.dram_tensor("out", [R, D], F32, kind="ExternalOutput")

    any_bias = any(has_bias)

    with tile.TileContext(nc) as tc, ExitStack() as ctx:
        cst = ctx.enter_context(tc.tile_pool(name="cst", bufs=1))
        wpool = ctx.enter_context(tc.tile_pool(name="wpool", bufs=w_bufs))
        htp = ctx.enter_context(tc.tile_pool(name="htp", bufs=8))
        atp = ctx.enter_context(tc.tile_pool(name="atp", bufs=at_bufs))
        xbp = ctx.enter_context(tc.tile_pool(name="xbp", bufs=3))
        smp = ctx.enter_context(tc.tile_pool(name="smp", bufs=4))
        obp = ctx.enter_context(tc.tile_pool(name="obp", bufs=3))
        afp = ctx.enter_context(tc.tile_pool(name="afp", bufs=2))
        accp = ctx.enter_context(tc.tile_pool(name="accp", bufs=8, space="PSUM"))

        ident = cst.tile([P, P], F32, tag="ident")
        make_identity(nc, ident[:])
        ident_bf = cst.tile([P, P], BF, tag="ident_bf")
        nc.scalar.activation(ident_bf[:], ident[:], AF.Copy)

        # gate_w [512, 8] -> [128, 4*8] (chunk ci at cols ci*8..)
        gw = cst.tile([P, NCH * K], BF, tag="gw")
        for c in range(NCH):
            nc.sync.dma_start(gw[:, c * K:(c + 1) * K], gate_w[c * P:(c + 1) * P, :])

        gb_bc = None
        ones_row = cst.tile([1, P], F32, tag="ones_row")
        nc.vector.memset(ones_row[:], 1.0)
        if has_gate_b:
            gb_row = cst.tile([1, K], F32, tag="gb_row")
            nc.sync.dma_start(gb_row[:], gate_b[None, :])
            gb_ps = accp.tile([P, CB], F32, tag="acc")
            nc.tensor.matmul(gb_ps[:, :K], ones_row[:], gb_row[:])
            gb_bc = cst.tile([P, K], F32, tag="gb_bc")
            nc.scalar.activation(gb_bc[:], gb_ps[:, :K], AF.Copy)

        bl_sb = [None] * 4
        alphaT_bf = None
        if any_bias:
            for li in range(4):
                if has_bias[li]:
                    blt = cst.tile([K, D], BF, tag=f"bl{li}")
                    nc.sync.dma_start(blt[:], bs[li][:, :])
                    bl_sb[li] = blt
            alphaT_bf = cst.tile([K, R], BF, tag="alphaT_bf")

        alpha = cst.tile([P, NT * K], F32, tag="alpha")
        alphaT = cst.tile([K, R], F32, tag="alphaT")
        alpha_bc = cst.tile([P, K, R], BF, tag="alpha_bc")

        # ---- prologue: x -> feature-major bf16 hT, gating, alpha ----
        hT = {}
        for c in range(NCH):
            hT[(0, c)] = htp.tile([P, R], BF, tag="ht", name=f"ht_0_{c}")
        for t in range(NT):
            xb = xbp.tile([P, D], F32, tag="xb")
            nc.sync.dma_start(xb[:], x[t * P:(t + 1) * P, :])
            tr = accp.tile([P, CB], F32, tag="acc", name=f"xtr_{t}")
            for c in range(NCH):
                nc.tensor.transpose(tr[:, c * P:(c + 1) * P], xb[:, c * P:(c + 1) * P], ident[:])
            for c in range(NCH):
                nc.scalar.activation(
                    hT[(0, c)][:, t * P:(t + 1) * P], tr[:, c * P:(c + 1) * P], AF.Copy
                )
            # gating logits for this tile (batch-major [128, 8])
            lg = accp.tile([P, CB], F32, tag="acc", name=f"lg_{t}")
            for c in range(NCH):
                nc.tensor.matmul(
                    lg[:, :K],
                    hT[(0, c)][:, t * P:(t + 1) * P],
                    gw[:, c * K:(c + 1) * K],
                    start=(c == 0),
                    stop=(c == NCH - 1),
                )
            ex = smp.tile([P, K], F32, tag="ex")
            if has_gate_b:
                nc.vector.scalar_tensor_tensor(
                    ex[:], lg[:, :K], 1.0, gb_bc[:], op0=ALU.mult, op1=ALU.add
                )
                nc.scalar.activation(ex[:], ex[:], AF.Exp)
            else:
                nc.scalar.activation(ex[:], lg[:, :K], AF.Exp)
            ssum = smp.tile([P, 1], F32, tag="ssum")
            nc.vector.reduce_sum(ssum[:], ex[:], axis=mybir.AxisListType.X)
            rec = smp.tile([P, 1], F32, tag="rec")
            nc.vector.reciprocal(rec[:], ssum[:])
            nc.vector.tensor_scalar_mul(alpha[:, t * K:(t + 1) * K], ex[:], rec[:])

            # alphaT[:, t*128:(t+1)*128] = alpha_tile.T (8 x 128)
            at_ps = accp.tile([P, CB], F32, tag="acc", name=f"atr_{t}")
            nc.tensor.transpose(at_ps[:K, :P], alpha[:, t * K:(t + 1) * K], ident[:])
            nc.scalar.activation(alphaT[:, t * P:(t + 1) * P], at_ps[:K, :P], AF.Copy)
            if any_bias:
                nc.scalar.activation(alphaT_bf[:, t * P:(t + 1) * P], at_ps[:K, :P], AF.Copy)

        # broadcast alpha across partitions: alpha_bc[k] = ones^T @ alphaT[k].
        # The moving operand must be based at partition 0/32/64, so each k row
        # is first DMA'd from partition k of alphaT onto a partition-0 tile.
        for k in range(K):
            arow = afp.tile([1, R], F32, tag="arow")
            nc.sync.dma_start(arow[:], alphaT[k:k + 1, :])
            for q in range(NPASS):
                bc = accp.tile([P, CB], F32, tag="acc", name=f"bc_{k}_{q}")
                nc.tensor.matmul(bc[:], ones_row[:], arow[0:1, q * CB:(q + 1) * CB])
                nc.scalar.activation(alpha_bc[:, k, q * CB:(q + 1) * CB], bc[:], AF.Copy)

        # ---- 4 MoIE blocks (x repeat for timing builds) ----
        for gli in range(4 * repeat):
            li = gli % 4
            last = gli == 4 * repeat - 1
            # stream this layer's weights (reused across all 4 passes)
            wt = {}
            for c in range(NCH):
                for k in range(K):
                    w_t = wpool.tile([P, D], BF, tag="w", name=f"w_{gli}_{c}_{k}")
                    nc.sync.dma_start(w_t[:], ws[li][k, c * P:(c + 1) * P, :])
                    wt[(c, k)] = w_t

            for co in range(NCH):
                hT[(gli + 1, co)] = htp.tile([P, R], BF, tag="ht", name=f"ht_{gli + 1}_{co}")

            use_bias = any_bias and has_bias[li]
            for ps in range(NPASS):
                cl = slice(ps * CB, (ps + 1) * CB)
                accs = [
                    accp.tile([P, CB], F32, tag="acc", name=f"acc_{gli}_{ps}_{co}")
                    for co in range(NCH)
                ]
                if use_bias:
                    for co in range(NCH):
                        nc.tensor.matmul(
                            accs[co][:],
                            bl_sb[li][:, co * P:(co + 1) * P],
                            alphaT_bf[:, cl],
                            start=True,
                            stop=False,
                        )
                at_same = None
                for c in range(NCH):
                    for k in range(K):
                        if mode == "pe_same" and at_same is not None:
                            at = at_same
                        else:
                            at = atp.tile([P, CB], BF, tag="at")
                            eng = nc.gpsimd if k >= K - gp_at else nc.vector
                            eng.tensor_mul(at[:], alpha_bc[:, k, cl], hT[(gli, c)][:, cl])
                            at_same = at
                        first = (c == 0 and k == 0) and not use_bias
                        for co in range(NCH):
                            nc.tensor.matmul(
                                accs[co][:],
                                wt[(c, k)][:, co * P:(co + 1) * P],
                                at[:],
                                start=first,
                                stop=(c == NCH - 1 and k == K - 1),
                            )
                for co in range(NCH):
                    nc.scalar.activation(
                        hT[(gli + 1, co)][:, cl], accs[co][:],
                        AF.Relu if (li < 3 and not last) else AF.Copy,
                    )

        # ---- epilogue: transpose back to batch-major and store ----
        final = hT[(4 * repeat, 0)], hT[(4 * repeat, 1)], hT[(4 * repeat, 2)], hT[(4 * repeat, 3)]
        for t in range(NT):
            trb = accp.tile([P, CB], BF, tag="acc", name=f"otr_{t}")
            for co in range(NCH):
                nc.tensor.transpose(
                    trb[:, co * P:(co + 1) * P],
                    final[co][:, t * P:(t + 1) * P],
                    ident_bf[:],
                )
            ob = obp.tile([P, D], F32, tag="ob")
            nc.scalar.activation(ob[:], trb[:], AF.Copy)
            nc.sync.dma_start(out[t * P:(t + 1) * P, :], ob[:])

    nc.compile()
    return nc


_CACHE = {}


def _get_nc(key):
    if key not in _CACHE:
        _CACHE[key] = _build(key[0], key[1])
    return _CACHE[key]


def _bf16(a):
    import ml_dtypes

    return np.ascontiguousarray(np.asarray(a, dtype=np.float32).astype(ml_dtypes.bfloat16))


def kernel(**inputs):
    x = np.ascontiguousarray(np.asarray(inputs["x"], dtype=np.float32))
    gate_w = _bf16(inputs["gate_w"])
    gate_b = np.ascontiguousarray(np.asarray(inputs["gate_b"], dtype=np.float32))
    wlist = [_bf16(inputs[n]) for n in W_NAMES]
    blist = [_bf16(inputs[n]) for n in B_NAMES]

    has_gate_b = bool(np.any(gate_b))
    has_bias = tuple(bool(np.any(np.asarray(inputs[n]))) for n in B_NAMES)
    nc = _get_nc((has_gate_b, has_bias))

    shared = {"gate_w": gate_w, "gate_b": gate_b}
    for n, w in zip(W_NAMES, wlist):
        shared[n] = w
    for n, b in zip(B_NAMES, blist):
        shared[n] = b

    core_ids = list(range(NCORES))
    in_maps = [dict(shared, x=x[i * R:(i + 1) * R]) for i in core_ids]
    res = run_bass_kernel_spmd(nc, in_maps, core_ids)
    return np.concatenate([res.results[i]["out"] for i in core_ids], axis=0)


if __name__ == "__main__":
    rng = np.random.default_rng(0)
    ins = {
        "x": rng.standard_normal((B, D), dtype=np.float32),
        "gate_w": rng.standard_normal((D, K), dtype=np.float32) * 0.02,
        "gate_b": np.zeros((K,), np.float32),
    }
    for n in W_NAMES:
        ins[n] = rng.standard_normal((K, D, D), dtype=np.float32) * 0.02
    for n in B_NAMES:
        ins[n] = np.zeros((K, D), np.float32)
    y = kernel(**ins)
    print("out", y.shape, y.dtype, float(np.abs(y).max()))


# revision 20
# speedup vs baseline: 1.0535x; 1.0535x over previous
"""Trainium2 Bass kernel for the BMoIE (dense mixture-of-experts) network.

Network (per sample):
    alpha = softmax(x @ gate_w + gate_b)                       # [B, 8]
    h = relu(sum_k alpha_k * (h @ w_l[k] + b_l[k]))            # 3 hidden blocks
    out = sum_k alpha_k * (h @ wo[k] + bo[k])                  # output block

Strategy: data-parallel over 8 NeuronCores (2048 rows each, SPMD, no
collectives). Per core, activations are kept FEATURE-MAJOR the whole way
(hT[ci] = [128 feat x 2048 batch]) and the per-sample alpha is folded into
the activations once per layer:

    AT[(k,ci)] = alpha_bc[k] * hT[ci]          (VectorE, bf16)
    outT[co]  += W[k,ci,co]^T @ AT[(k,ci)]     (PE, accumulate in PSUM)

which turns each MoIE block into ONE dense GEMM with contraction K*D=4096
(32 accumulation steps per PSUM bank) — no per-expert combine pass and no
transposes between layers. ReLU is fused into the PSUM->SBUF eviction
(ScalarE), which writes the next layer's bf16 hT directly.

The layer is processed in 4 column-passes of 512 batch columns; each pass
accumulates the 4 output chunks in 4 PSUM banks while the previous pass's
4 banks drain through ScalarE, so the PE stream never waits on evictions
(avoids the HAM clock-gate re-throttle that cost the batch-major version
~120us). All matmul operands are bf16 (full PE speed; LDWEIGHTS for a
bf16 stationary is 64 cyc and hidden behind the 512-cycle matmuls).

alpha is computed batch-major (baseline gating code), transposed to
alphaT[8, 2048] with PE transposes, then broadcast across partitions with
ones[1,128]^T @ alphaT[k] PE matmuls into alpha_bc[k] = [128, 2048] bf16.
Per-layer biases (zero in the graded problem, supported anyway) enter as
the first accumulation step: acc[co] = bl[li][:, co]^T @ alphaT_bf.
"""

import sys

sys.path.insert(0, "/opt/trn_rl_repo")

import numpy as np

import concourse.bass as bass
import concourse.mybir as mybir
import concourse.tile as tile
from concourse import bacc
from concourse.bass_utils import run_bass_kernel_spmd
from concourse.masks import make_identity

P = 128           # partitions
D = 512           # model dim (= hidden dim)
K = 8             # experts
NCORES = 8
B = 16384
R = B // NCORES   # rows per core (batch columns in feature-major layout)
NT = R // P       # 16 batch tiles per core
NCH = D // P      # 4 feature chunks (both input ci and output co)
CB = 512          # batch columns per pass (PSUM bank = 512 fp32)
NPASS = R // CB   # 4 passes per layer
F32 = mybir.dt.float32
BF = mybir.dt.bfloat16
AF = mybir.ActivationFunctionType
ALU = mybir.AluOpType

W_NAMES = ("w0", "w1", "w2", "wo")
B_NAMES = ("b0", "b1", "b2", "bo")


def _build(has_gate_b, has_bias, repeat=1, mode="full", at_bufs=8, w_bufs=64,
           gp_at=0, dma_split=False):
    """Trace + compile the per-core kernel. has_bias is a 4-tuple of bools.

    repeat>1 runs the whole 4-layer stack that many times (same weights,
    full DMA traffic each time) — used only for timing measurements.
    mode: "full" | "pe_same" (AT generated once per pass and reused by all
    32 accumulation steps — isolates PE throughput; wrong results).
    gp_at: number of experts (out of 8) whose AT scaling runs on GpSimd
    instead of VectorE.
    """
    from contextlib import ExitStack

    nc = bacc.Bacc("TRN2", target_bir_lowering=False, num_devices=NCORES)
    x = nc.dram_tensor("x", [R, D], F32, kind="ExternalInput")
    gate_w = nc.dram_tensor("gate_w", [D, K], BF, kind="ExternalInput")
    gate_b = nc.dram_tensor("gate_b", [K], F32, kind="ExternalInput")
    ws = [nc.dram_tensor(n, [K, D, D], BF, kind="ExternalInput") for n in W_NAMES]
    bs = [nc.dram_tensor(n, [K, D], BF, kind="ExternalInput") for n in B_NAMES]
    out = nc.dram_tensor("out", [R, D], F32, kind="ExternalOutput")

    any_bias = any(has_bias)

    if mode == "trivial":
        with tile.TileContext(nc) as tc, ExitStack() as ctx:
            pool = ctx.enter_context(tc.tile_pool(name="triv", bufs=2))
            tt = pool.tile([P, D], F32, tag="tt")
            nc.sync.dma_start(tt[:], x[0:P, :])
            nc.sync.dma_start(out[0:P, :], tt[:])
        nc.compile()
        return nc

    with tile.TileContext(nc) as tc, ExitStack() as ctx:
        cst = ctx.enter_context(tc.tile_pool(name="cst", bufs=1))
        wpool = ctx.enter_context(tc.tile_pool(name="wpool", bufs=w_bufs))
        htp = ctx.enter_context(tc.tile_pool(name="htp", bufs=8))
        atp = ctx.enter_context(tc.tile_pool(name="atp", bufs=at_bufs))
        xbp = ctx.enter_context(tc.tile_pool(name="xbp", bufs=3))
        smp = ctx.enter_context(tc.tile_pool(name="smp", bufs=4))
        obp = ctx.enter_context(tc.tile_pool(name="obp", bufs=3))
        afp = ctx.enter_context(tc.tile_pool(name="afp", bufs=2))
        accp = ctx.enter_context(tc.tile_pool(name="accp", bufs=8, space="PSUM"))

        ident = cst.tile([P, P], F32, tag="ident")
        make_identity(nc, ident[:])
        ident_bf = cst.tile([P, P], BF, tag="ident_bf")
        nc.scalar.activation(ident_bf[:], ident[:], AF.Copy)

        # gate_w [512, 8] -> [128, 4*8] (chunk ci at cols ci*8..)
        gw = cst.tile([P, NCH * K], BF, tag="gw")
        for c in range(NCH):
            nc.sync.dma_start(gw[:, c * K:(c + 1) * K], gate_w[c * P:(c + 1) * P, :])

        gb_bc = None
        ones_row = cst.tile([1, P], F32, tag="ones_row")
        nc.vector.memset(ones_row[:], 1.0)
        if has_gate_b:
            gb_row = cst.tile([1, K], F32, tag="gb_row")
            nc.sync.dma_start(gb_row[:], gate_b[None, :])
            gb_ps = accp.tile([P, CB], F32, tag="acc")
            nc.tensor.matmul(gb_ps[:, :K], ones_row[:], gb_row[:])
            gb_bc = cst.tile([P, K], F32, tag="gb_bc")
            nc.scalar.activation(gb_bc[:], gb_ps[:, :K], AF.Copy)

        bl_sb = [None] * 4
        alphaT_bf = None
        if any_bias:
            for li in range(4):
                if has_bias[li]:
                    blt = cst.tile([K, D], BF, tag=f"bl{li}")
                    nc.sync.dma_start(blt[:], bs[li][:, :])
                    bl_sb[li] = blt
            alphaT_bf = cst.tile([K, R], BF, tag="alphaT_bf")

        alpha = cst.tile([P, NT * K], F32, tag="alpha")
        alphaT = cst.tile([K, R], F32, tag="alphaT")
        alpha_bc = cst.tile([P, K, R], BF, tag="alpha_bc")

        # ---- prologue: x -> feature-major bf16 hT, gating, alpha ----
        hT = {}
        for c in range(NCH):
            hT[(0, c)] = htp.tile([P, R], BF, tag="ht", name=f"ht_0_{c}")
        for t in range(NT):
            xb = xbp.tile([P, D], F32, tag="xb")
            nc.sync.dma_start(xb[:], x[t * P:(t + 1) * P, :])
            tr = accp.tile([P, CB], F32, tag="acc", name=f"xtr_{t}")
            for c in range(NCH):
                nc.tensor.transpose(tr[:, c * P:(c + 1) * P], xb[:, c * P:(c + 1) * P], ident[:])
            for c in range(NCH):
                nc.scalar.activation(
                    hT[(0, c)][:, t * P:(t + 1) * P], tr[:, c * P:(c + 1) * P], AF.Copy
                )
            # gating logits for this tile (batch-major [128, 8])
            lg = accp.tile([P, CB], F32, tag="acc", name=f"lg_{t}")
            for c in range(NCH):
                nc.tensor.matmul(
                    lg[:, :K],
                    hT[(0, c)][:, t * P:(t + 1) * P],
                    gw[:, c * K:(c + 1) * K],
                    start=(c == 0),
                    stop=(c == NCH - 1),
                )
            ex = smp.tile([P, K], F32, tag="ex")
            if has_gate_b:
                nc.vector.scalar_tensor_tensor(
                    ex[:], lg[:, :K], 1.0, gb_bc[:], op0=ALU.mult, op1=ALU.add
                )
                nc.scalar.activation(ex[:], ex[:], AF.Exp)
            else:
                nc.scalar.activation(ex[:], lg[:, :K], AF.Exp)
            ssum = smp.tile([P, 1], F32, tag="ssum")
            nc.vector.reduce_sum(ssum[:], ex[:], axis=mybir.AxisListType.X)
            rec = smp.tile([P, 1], F32, tag="rec")
            nc.vector.reciprocal(rec[:], ssum[:])
            nc.vector.tensor_scalar_mul(alpha[:, t * K:(t + 1) * K], ex[:], rec[:])

            # alphaT[:, t*128:(t+1)*128] = alpha_tile.T (8 x 128)
            at_ps = accp.tile([P, CB], F32, tag="acc", name=f"atr_{t}")
            nc.tensor.transpose(at_ps[:K, :P], alpha[:, t * K:(t + 1) * K], ident[:])
            nc.scalar.activation(alphaT[:, t * P:(t + 1) * P], at_ps[:K, :P], AF.Copy)
            if any_bias:
                nc.scalar.activation(alphaT_bf[:, t * P:(t + 1) * P], at_ps[:K, :P], AF.Copy)

        # broadcast alpha across partitions: alpha_bc[k] = ones^T @ alphaT[k].
        # The moving operand must be based at partition 0/32/64, so each k row
        # is first DMA'd from partition k of alphaT onto a partition-0 tile.
        for k in range(K):
            arow = afp.tile([1, R], F32, tag="arow")
            nc.sync.dma_start(arow[:], alphaT[k:k + 1, :])
            for q in range(NPASS):
                bc = accp.tile([P, CB], F32, tag="acc", name=f"bc_{k}_{q}")
                nc.tensor.matmul(bc[:], ones_row[:], arow[0:1, q * CB:(q + 1) * CB])
                nc.scalar.activation(alpha_bc[:, k, q * CB:(q + 1) * CB], bc[:], AF.Copy)

        # ---- 4 MoIE blocks (x repeat for timing builds) ----
        pe_pure = mode in ("pe_pure", "pe_pure_cycle", "pe_evict", "pe_dma")
        pp_evict = mode == "pe_evict"
        pp_dma = mode == "pe_dma"
        if pe_pure:
            # timing probe: zero cross-engine deps in the loop. One static AT
            # tile, layer-0 weights only, no evictions. Results are garbage.
            at_st = cst.tile([P, CB], BF, tag="at_st")
            nc.vector.memset(at_st[:], 0.125)
        wt0 = None
        for gli in range(4 * repeat):
            li = gli % 4
            last = gli == 4 * repeat - 1
            # stream this layer's weights (reused across all 4 passes)
            if pe_pure and not pp_dma and wt0 is not None:
                wt = wt0
            else:
                wt = {}
                for c in range(NCH):
                    for k in range(K):
                        w_t = wpool.tile([P, D], BF, tag="w", name=f"w_{gli}_{c}_{k}")
                        deng = nc.scalar if (dma_split and (c * K + k) % 2) else nc.sync
                        deng.dma_start(w_t[:], ws[li][k, c * P:(c + 1) * P, :])
                        wt[(c, k)] = w_t
                wt0 = wt
            if pe_pure:
                if pp_evict:
                    for co in range(NCH):
                        hT[(gli + 1, co)] = htp.tile([P, R], BF, tag="ht",
                                                     name=f"ht_{gli + 1}_{co}")
                for ps in range(NPASS):
                    if mode != "pe_pure_cycle":
                        for co in range(NCH):
                            acc = accp.tile([P, CB], F32, tag="acc",
                                            name=f"acc_{gli}_{ps}_{co}")
                            for i in range(NCH * K):
                                c, k = divmod(i, K)
                                nc.tensor.matmul(
                                    acc[:], wt[(c, k)][:, co * P:(co + 1) * P],
                                    at_st[:], start=(i == 0), stop=(i == NCH * K - 1),
                                )
                            if pp_evict:
                                nc.scalar.activation(
                                    hT[(gli + 1, co)][:, ps * CB:(ps + 1) * CB],
                                    acc[:], AF.Relu,
                                )
                    else:
                        accs = [accp.tile([P, CB], F32, tag="acc",
                                          name=f"acc_{gli}_{ps}_{co}")
                                for co in range(NCH)]
                        for i in range(NCH * K):
                            c, k = divmod(i, K)
                            for co in range(NCH):
                                nc.tensor.matmul(
                                    accs[co][:], wt[(c, k)][:, co * P:(co + 1) * P],
                                    at_st[:], start=(i == 0), stop=(i == NCH * K - 1),
                                )
                if last:
                    # minimal epilogue so the output tensor is written
                    for t in range(NT):
                        ob = obp.tile([P, D], F32, tag="ob")
                        nc.scalar.activation(ob[:], at_st[:, :D] if CB >= D else at_st[:], AF.Copy)
                        nc.sync.dma_start(out[t * P:(t + 1) * P, :], ob[:])
                continue

            for co in range(NCH):
                hT[(gli + 1, co)] = htp.tile([P, R], BF, tag="ht", name=f"ht_{gli + 1}_{co}")

            use_bias = any_bias and has_bias[li]
            act_f = AF.Relu if (li < 3 and not last) else AF.Copy
            for ps in range(NPASS):
                cl = slice(ps * CB, (ps + 1) * CB)
                if mode == "corun":
                    # generate all 32 AT tiles for this pass, then give each
                    # PSUM bank a contiguous 32-matmul accumulation run (no
                    # per-matmul bank cycling; groups close staggered so
                    # evictions overlap the next bank's run)
                    ats = []
                    for c in range(NCH):
                        for k in range(K):
                            at = atp.tile([P, CB], BF, tag="at")
                            eng = nc.gpsimd if k >= K - gp_at else nc.vector
                            eng.tensor_mul(at[:], alpha_bc[:, k, cl], hT[(gli, c)][:, cl])
                            ats.append(at)
                    for co in range(NCH):
                        acc = accp.tile([P, CB], F32, tag="acc",
                                        name=f"acc_{gli}_{ps}_{co}")
                        if use_bias:
                            nc.tensor.matmul(
                                acc[:], bl_sb[li][:, co * P:(co + 1) * P],
                                alphaT_bf[:, cl], start=True, stop=False,
                            )
                        for i in range(NCH * K):
                            c, k = divmod(i, K)
                            nc.tensor.matmul(
                                acc[:], wt[(c, k)][:, co * P:(co + 1) * P], ats[i][:],
                                start=(i == 0 and not use_bias),
                                stop=(i == NCH * K - 1),
                            )
                        nc.scalar.activation(hT[(gli + 1, co)][:, cl], acc[:], act_f)
                    continue
                accs = [
                    accp.tile([P, CB], F32, tag="acc", name=f"acc_{gli}_{ps}_{co}")
                    for co in range(NCH)
                ]
                if use_bias:
                    for co in range(NCH):
                        nc.tensor.matmul(
                            accs[co][:],
                            bl_sb[li][:, co * P:(co + 1) * P],
                            alphaT_bf[:, cl],
                            start=True,
                            stop=False,
                        )
                at_same = None
                for c in range(NCH):
                    for k in range(K):
                        if mode == "pe_same" and at_same is not None:
                            at = at_same
                        else:
                            at = atp.tile([P, CB], BF, tag="at")
                            eng = nc.gpsimd if k >= K - gp_at else nc.vector
                            eng.tensor_mul(at[:], alpha_bc[:, k, cl], hT[(gli, c)][:, cl])
                            at_same = at
                        first = (c == 0 and k == 0) and not use_bias
                        for co in range(NCH):
                            nc.tensor.matmul(
                                accs[co][:],
                                wt[(c, k)][:, co * P:(co + 1) * P],
                                at[:],
                                start=first,
                                stop=(c == NCH - 1 and k == K - 1),
                            )
                for co in range(NCH):
                    nc.scalar.activation(
                        hT[(gli + 1, co)][:, cl], accs[co][:],
                        AF.Relu if (li < 3 and not last) else AF.Copy,
                    )
                if last and mode == "full":
                    # inline epilogue: transpose this pass's columns back to
                    # batch-major while later passes still compute
                    for tb in range(CB // P):
                        t = ps * (CB // P) + tb
                        trb = accp.tile([P, CB], BF, tag="acc", name=f"otr_{t}")
                        for co in range(NCH):
                            nc.tensor.transpose(
                                trb[:, co * P:(co + 1) * P],
                                hT[(gli + 1, co)][:, t * P:(t + 1) * P],
                                ident_bf[:],
                            )
                        ob = obp.tile([P, D], F32, tag="ob")
                        nc.scalar.activation(ob[:], trb[:], AF.Copy)
                        nc.sync.dma_start(out[t * P:(t + 1) * P, :], ob[:])

        # ---- epilogue: transpose back to batch-major and store ----
        if not pe_pure and mode != "full":
            final = [hT[(4 * repeat, co)] for co in range(NCH)]
            for t in range(NT):
                trb = accp.tile([P, CB], BF, tag="acc", name=f"otr_{t}")
                for co in range(NCH):
                    nc.tensor.transpose(
                        trb[:, co * P:(co + 1) * P],
                        final[co][:, t * P:(t + 1) * P],
                        ident_bf[:],
                    )
                ob = obp.tile([P, D], F32, tag="ob")
                nc.scalar.activation(ob[:], trb[:], AF.Copy)
                nc.sync.dma_start(out[t * P:(t + 1) * P, :], ob[:])

    nc.compile()
    return nc


_CACHE = {}


def _get_nc(key):
    if key not in _CACHE:
        _CACHE[key] = _build(key[0], key[1])
    return _CACHE[key]


def _bf16(a):
    import ml_dtypes

    return np.ascontiguousarray(np.asarray(a, dtype=np.float32).astype(ml_dtypes.bfloat16))


def kernel(**inputs):
    x = np.ascontiguousarray(np.asarray(inputs["x"], dtype=np.float32))
    gate_w = _bf16(inputs["gate_w"])
    gate_b = np.ascontiguousarray(np.asarray(inputs["gate_b"], dtype=np.float32))
    wlist = [_bf16(inputs[n]) for n in W_NAMES]
    blist = [_bf16(inputs[n]) for n in B_NAMES]

    has_gate_b = bool(np.any(gate_b))
    has_bias = tuple(bool(np.any(np.asarray(inputs[n]))) for n in B_NAMES)
    nc = _get_nc((has_gate_b, has_bias))

    shared = {"gate_w": gate_w, "gate_b": gate_b}
    for n, w in zip(W_NAMES, wlist):
        shared[n] = w
    for n, b in zip(B_NAMES, blist):
        shared[n] = b

    core_ids = list(range(NCORES))
    in_maps = [dict(shared, x=x[i * R:(i + 1) * R]) for i in core_ids]
    res = run_bass_kernel_spmd(nc, in_maps, core_ids)
    return np.concatenate([res.results[i]["out"] for i in core_ids], axis=0)


if __name__ == "__main__":
    rng = np.random.default_rng(0)
    ins = {
        "x": rng.standard_normal((B, D), dtype=np.float32),
        "gate_w": rng.standard_normal((D, K), dtype=np.float32) * 0.02,
        "gate_b": np.zeros((K,), np.float32),
    }
    for n in W_NAMES:
        ins[n] = rng.standard_normal((K, D, D), dtype=np.float32) * 0.02
    for n in B_NAMES:
        ins[n] = np.zeros((K, D), np.float32)
    y = kernel(**ins)
    print("out", y.shape, y.dtype, float(np.abs(y).max()))
